# revision 31
# baseline (speedup 1.0000x reference)
"""Causal self-attention (B=2, T=4096, C=768, H=12, D=64) on 8 Trainium2 cores.

Sharding: (batch, head-group) across 8 cores — core i handles batch i//4,
heads 3*(i%4) .. 3*(i%4)+2.  Each core computes q/k in a transposed [d, T]
layout (S^T formulation: no transposes anywhere in attention), v in natural
[T, d] layout packed next to a ones-block so one AV matmul produces both
y_un^T and the broadcast softmax denominator.  Output projection produces a
partial z[T, C] per core; host sums the 4 partials per batch and adds biases.

S matmuls contract over D=64 only, so they are packed pairwise onto the two
64-row PE tiles (tile_position auto-derived from base_partition 0/64): head
pair (h0 lo, h1 hi) shares each kt-column block, and h2 is duplicated onto
both halves so its even/odd kt blocks run concurrently too.  This halves the
PE streaming time of the S phase.  qT/kT layout [128, 2, T]: slab 0 =
[q0;q1] straight from the QKV psum (no partition shifts), slab 1 = h2
duplicated via one staged shift DMA pair per chunk.

Numerics: all matmuls in fp16 (same 10-bit mantissa as TF32/fp32r, but full
PE rate), fp32 PSUM accumulation.  Softmax exp has no max-subtraction; a
constant exp(S-10) shift keeps probs inside fp16 range and cancels in the
normalization.  v-bias and output bias fold into one host-side row:
y @ W_p + b_p == (y0/rowsum) @ W_p + (b_v @ W_p + b_p).

Perf notes (measured on HW): fp16/bf16 matmul N=512 is ~222 ns warm; paired
K=64 matmuls on row groups 0/64 run concurrently (verified exact); matmuls
that alternate contraction row-group config (K=64 tiles vs K=128 full) pay
~100 ns per transition, so S-runs and AV-runs are batched ~4 units long.
reciprocal_approx_fast is ~5x reciprocal but silently broken at
base_partition 64 — always call it over the full 128 partitions.
"""
import os
import sys

sys.path.insert(0, "/opt/trn_rl_repo")

import numpy as np

B, T, C = 2, 4096, 768
H, D = 12, 64
HPC = 3            # heads per core
NCORE = 8
QC = 512           # q-chunk (free dim of S^T blocks)
KTS = 128          # k-tile size
NJQ = T // QC      # 8 q-chunks
NKT = T // KTS     # 32 k-tiles
NTT = T // 128     # 32 t-tiles (proj)
NCCH = C // 128    # 6 contraction chunks

# vones column layout: [v0 | ones | v1 | v2 | ones]
VONES_W = 320
EXP_SHIFT = -10.0

_cache = {}
last_results = None  # set by kernel(); test.py reads exec_time_ns off this


def _build():
    import concourse.mybir as mybir
    import concourse.tile as tile
    from concourse import bacc

    F32 = mybir.dt.float32
    F16 = mybir.dt.float16
    AF = mybir.ActivationFunctionType

    nc = bacc.Bacc("TRN2", target_bir_lowering=False, debug=False)

    xT = nc.dram_tensor("xT", [C, T], F16, kind="ExternalInput").ap()
    wqk = nc.dram_tensor("wqk", [C, 384], F16, kind="ExternalInput").ap()
    wv = nc.dram_tensor("wv", [C, 192], F16, kind="ExternalInput").ap()
    wp = nc.dram_tensor("wp", [192, C], F16, kind="ExternalInput").ap()
    bqk = nc.dram_tensor("bqk", [128, 3], F32, kind="ExternalInput").ap()
    trimask = nc.dram_tensor("trimask", [128, 128], F16, kind="ExternalInput").ap()
    z = nc.dram_tensor("z", [T, C], F16, kind="ExternalOutput").ap()
    debug = os.environ.get("CC_ATTN_DEBUG", "0") == "1"
    if debug:
        dq = nc.dram_tensor("dbg_qT", [128, 2, T], F16, kind="ExternalOutput").ap()
        dk = nc.dram_tensor("dbg_kT", [128, 2, T], F16, kind="ExternalOutput").ap()
        dv = nc.dram_tensor("dbg_vones", [128, 32 * VONES_W], F16,
                            kind="ExternalOutput").ap()
        dy0 = nc.dram_tensor("dbg_yT0", [128, T], F16, kind="ExternalOutput").ap()
        dy1 = nc.dram_tensor("dbg_yT1", [64, T], F16, kind="ExternalOutput").ap()

    with tile.TileContext(nc) as tc:
        with tc.tile_pool(name="persist", bufs=1) as persist:
            qT = persist.tile([128, 2, T], F16, tag="qT")
            kT = persist.tile([128, 2, T], F16, tag="kT")
            vones = persist.tile([128, NKT, VONES_W], F16, tag="vones")
            yT0 = persist.tile([128, T], F16, tag="yT0")
            yT1 = persist.tile([64, T], F16, tag="yT1")
            bqk_sb = persist.tile([128, 3], F32, tag="bqk")
            shift_sb = persist.tile([128, 1], F32, tag="shift")
            tri_sb = persist.tile([128, 128], F16, tag="tri")

            nc.sync.dma_start(bqk_sb[:], bqk)
            nc.sync.dma_start(tri_sb[:], trimask)
            nc.vector.memset(shift_sb[:], EXP_SHIFT)
            # only the ones-blocks need init; v blocks are written by phase A
            nc.vector.memset(vones[:, :, 64:128], 1.0)
            nc.vector.memset(vones[:, :, 256:320], 1.0)

            # ---- Interleaved pipeline: A(tch) then B(jq=tch), D at end ----
            # wqk columns: [q0 q1 | k0 k1 | q2 k2]; mt0/mt1 psums map straight
            # onto qT/kT slab 0; mt2 is staged and duplicated onto both
            # partition halves of slab 1 (one DVE copy + one shift DMA each).
            with (
                tc.tile_pool(name="aw", bufs=1) as aw,
                tc.tile_pool(name="ax", bufs=2) as ax,
                tc.tile_pool(name="ast", bufs=3) as ast,
                tc.tile_pool(name="dz", bufs=3) as dz,
                tc.tile_pool(name="bexp", bufs=9) as bexp,
                tc.tile_pool(name="bst", bufs=6) as bst,
                tc.tile_pool(name="bpsS", bufs=3, space="PSUM") as bpsS,
                tc.tile_pool(name="bpsY", bufs=2, space="PSUM") as bpsY,
            ):
                wqk_sb = aw.tile([128, NCCH, 384], F16, tag="wqk")
                wv_sb = aw.tile([128, NCCH, 192], F16, tag="wv")
                wp0_sb = aw.tile([128, C], F16, tag="wp0")
                wp1_sb = aw.tile([64, C], F16, tag="wp1")
                for ko in range(NCCH):
                    nc.sync.dma_start(wqk_sb[:, ko, :],
                                      wqk[ko * 128:(ko + 1) * 128, :])
                nc.sync.dma_start(wv_sb[:], wv.rearrange("(ko p) m -> p ko m", p=128))
                nc.sync.dma_start(wp0_sb[:], wp[0:128, :])
                nc.sync.dma_start(wp1_sb[:], wp[128:192, :])

                # PE warm-up burst: ~7us of dummy matmuls on scratch data so
                # the HAM clock-gate reaches 8/8 while the first input DMAs
                # are still in flight (PE would otherwise sit idle and start
                # phase A at 1.2 GHz).
                warm = aw.tile([128, QC], F16, tag="warm")
                nc.vector.memset(warm[:], 0.5)
                for wi in range(12):
                    wps = bpsS.tile([128, 1024], F32, tag="psS",
                                    name=f"warm{wi}")[:, 0:QC]
                    nc.tensor.matmul(wps[:], warm[:, (wi % 4) * 128:(wi % 4) * 128 + 128],
                                     warm[:], start=True, stop=True)

                def make_A_groups(tch):
                    tcols = slice(tch * QC, (tch + 1) * QC)
                    xslab = ax.tile([128, NCCH, QC], F16, tag="xslab")
                    for ko in range(NCCH):
                        nc.sync.dma_start(xslab[:, ko, :],
                                          xT[ko * 128:(ko + 1) * 128, tcols])

                    def mk_qk(mt):
                        def g():
                            ps = bpsS.tile([128, 1024], F32, tag="psS",
                                           name=f"psA{tch}_{mt}")[:, 0:QC]
                            for cch in range(NCCH):
                                nc.tensor.matmul(
                                    ps[:], wqk_sb[:, cch, mt * 128:(mt + 1) * 128],
                                    xslab[:, cch, :],
                                    start=(cch == 0), stop=(cch == NCCH - 1))
                            if mt == 0:
                                nc.vector.tensor_scalar_add(qT[:, 0, tcols], ps[:],
                                                            bqk_sb[:, 0:1])
                            elif mt == 1:
                                nc.vector.tensor_scalar_add(kT[:, 0, tcols], ps[:],
                                                            bqk_sb[:, 1:2])
                            else:
                                # mt2 psum rows: 0:64 = q2, 64:128 = k2.
                                stg = ast.tile([128, QC], F16, tag="astg")
                                nc.vector.tensor_scalar_add(stg[:], ps[:],
                                                            bqk_sb[:, 2:3])
                                nc.vector.tensor_copy(qT[0:64, 1, tcols],
                                                      stg[0:64, :])
                                nc.vector.tensor_copy(kT[64:128, 1, tcols],
                                                      stg[64:128, :])
                                nc.sync.dma_start(qT[64:128, 1, tcols], stg[0:64, :])
                                nc.sync.dma_start(kT[0:64, 1, tcols], stg[64:128, :])
                        return g

                    def mk_v(sub):
                        def g():
                            psv = bpsS.tile([128, 1024], F32, tag="psS",
                                            name=f"psV{tch}_{sub}")[:, 0:QC]
                            for cch in range(NCCH):
                                nc.tensor.matmul(
                                    psv[:, 0:192],
                                    xslab[:, cch, sub * 128:(sub + 1) * 128],
                                    wv_sb[:, cch, :],
                                    start=(cch == 0), stop=(cch == NCCH - 1))
                            tt = tch * 4 + sub
                            nc.vector.tensor_copy(vones[:, tt, 0:64], psv[:, 0:64])
                            nc.vector.tensor_copy(vones[:, tt, 128:256],
                                                  psv[:, 64:192])
                        return g

                    return [mk_qk(mt) for mt in range(3)] + [mk_v(s) for s in range(4)]

                def make_proj(tt):
                    def g():
                        tsl = slice(tt * 128, (tt + 1) * 128)
                        pz = bpsS.tile([128, 1024], F32, tag="psS", name=f"pz{tt}")
                        nc.tensor.matmul(pz[:, 0:512], yT0[:, tsl], wp0_sb[:, 0:512],
                                         start=True, stop=False)
                        nc.tensor.matmul(pz[:, 512:768], yT0[:, tsl],
                                         wp0_sb[:, 512:768], start=True, stop=False)
                        nc.tensor.matmul(pz[:, 0:512], yT1[:, tsl], wp1_sb[:, 0:512],
                                         start=False, stop=True)
                        nc.tensor.matmul(pz[:, 512:768], yT1[:, tsl],
                                         wp1_sb[:, 512:768], start=False, stop=True)
                        zt = dz.tile([128, C], F16, tag="zt")
                        nc.vector.tensor_copy(zt[:], pz[:, 0:C])
                        nc.sync.dma_start(z[tsl, :], zt[:])
                    return g

                def emit_B(jq, inserts, tail_prev=None):
                    nkt = 4 * jq + 4
                    nu2 = nkt // 2
                    total_units = nkt + nu2
                    spacing = max(1, total_units // max(len(inserts), 1))
                    gu = [0]

                    def qsl(off):
                        return slice(jq * QC + off, (jq + 1) * QC)

                    def ktsl(kt):
                        return slice(kt * KTS, (kt + 1) * KTS)

                    es_P = [None] * nkt
                    es_H = [None] * nu2
                    psY = {}

                    def s_unit(u):
                        kind, i = u
                        ps = bpsS.tile([128, 1024], F32, tag="psS")
                        es = bexp.tile([128, 1024], F16, tag="es")
                        if kind == "P":
                            kt = i
                            r = kt - 4 * jq
                            off = max(r, 0) * KTS
                            w = QC - off
                            # lo = h0 in bank 0, hi = h1 at fixed offset 512
                            # (bank 1) so no MM crosses a PSUM bank boundary.
                            nc.tensor.matmul(ps[:, 0:w], kT[0:64, 0, ktsl(kt)],
                                             qT[0:64, 0, qsl(off)],
                                             start=True, stop=True)
                            nc.tensor.matmul(ps[:, 512:512 + w],
                                             kT[64:128, 0, ktsl(kt)],
                                             qT[64:128, 0, qsl(off)],
                                             start=True, stop=True)
                            # one exp for both heads; diag units use a
                            # strided source AP that skips the unwritten
                            # [w:512] psum gap.
                            if w == QC:
                                nc.scalar.activation(es[:, 0:1024], ps[:, 0:1024],
                                                     AF.Exp, bias=shift_sb[:, 0:1])
                            else:
                                esrc = ps[:, 0:1024].rearrange(
                                    "p (u w) -> p u w", u=2)[:, :, 0:w]
                                edst = es[:, 0:2 * w].rearrange(
                                    "p (u w) -> p u w", u=2)
                                nc.scalar.activation(edst, esrc, AF.Exp,
                                                     bias=shift_sb[:, 0:1])
                            if r >= 0:
                                nc.vector.tensor_mul(es[:, 0:128], es[:, 0:128],
                                                     tri_sb[:])
                                nc.vector.tensor_mul(es[:, w:w + 128],
                                                     es[:, w:w + 128], tri_sb[:])
                            es_P[kt] = (es, off, w)
                        else:
                            # sub 0 (lo tile) -> bank 0 at col 0; sub 1 (hi
                            # tile) -> bank 1 at col 512.  Concurrent row
                            # tiles must never touch the same PSUM bank, so
                            # when both kt fit in one bank (last diag pair)
                            # run both serially on the lo tile instead.
                            kts = [2 * i, 2 * i + 1]
                            ws = [QC - max(kt - 4 * jq, 0) * KTS for kt in kts]
                            one_bank = ws[0] + ws[1] <= 512
                            offs = []
                            pos = 0
                            for sub, (kt, w) in enumerate(zip(kts, ws)):
                                off = QC - w
                                base = 0 if one_bank else sub * 64
                                nc.tensor.matmul(
                                    ps[:, pos:pos + w],
                                    kT[base:base + 64, 1, ktsl(kt)],
                                    qT[base:base + 64, 1, qsl(off)],
                                    start=True, stop=True)
                                offs.append((kt, off, w, pos))
                                pos = (pos + w) if one_bank else 512
                            w0, w1 = ws
                            if one_bank:
                                nc.scalar.activation(es[:, 0:w0 + w1],
                                                     ps[:, 0:w0 + w1], AF.Exp,
                                                     bias=shift_sb[:, 0:1])
                            elif w0 == QC:
                                nc.scalar.activation(es[:, 0:512 + w1],
                                                     ps[:, 0:512 + w1], AF.Exp,
                                                     bias=shift_sb[:, 0:1])
                            else:
                                nc.scalar.activation(es[:, 0:w0], ps[:, 0:w0],
                                                     AF.Exp, bias=shift_sb[:, 0:1])
                                nc.scalar.activation(es[:, 512:512 + w1],
                                                     ps[:, 512:512 + w1], AF.Exp,
                                                     bias=shift_sb[:, 0:1])
                            for kt, off, w, p_ in offs:
                                if kt >= 4 * jq:
                                    nc.vector.tensor_mul(es[:, p_:p_ + 128],
                                                         es[:, p_:p_ + 128],
                                                         tri_sb[:])
                            es_H[i] = (es, offs)

                    def av_unit(u):
                        kind, i = u
                        if kind == "P":
                            if "lo" not in psY:
                                psY["lo"] = bpsY.tile([128, QC], F32, tag="psY",
                                                      name=f"psY{jq}lo")
                                psY["hi"] = bpsY.tile([128, QC], F32, tag="psY",
                                                      name=f"psY{jq}hi")
                            es, off, w = es_P[i]
                            nc.tensor.matmul(psY["lo"][:, off:QC],
                                             vones[:, i, 0:128], es[:, 0:w],
                                             start=(i == 0), stop=(i == nkt - 1))
                            nc.tensor.matmul(psY["hi"][:, off:QC],
                                             vones[:, i, 64:192],
                                             es[:, w:2 * w],
                                             start=(i == 0), stop=(i == nkt - 1))
                            es_P[i] = None
                        else:
                            if "h2" not in psY:
                                psY["h2"] = bpsY.tile([128, QC], F32, tag="psY",
                                                      name=f"psY{jq}h2")
                            es, offs = es_H[i]
                            for j, (kt, off, w, p_) in enumerate(offs):
                                nc.tensor.matmul(psY["h2"][:, off:QC],
                                                 vones[:, kt, 192:320],
                                                 es[:, p_:p_ + w],
                                                 start=(i == 0 and j == 0),
                                                 stop=(i == nu2 - 1 and j == 1))
                            es_H[i] = None

                    def norm(h, psY_t):
                        # one full-width stage copy frees the psY bank fast;
                        # out-of-place fast reciprocal over all 128 partitions
                        # (base-partition 64 variant is broken); the y-half
                        # recip output is garbage and unused.
                        ystage = bst.tile([128, QC], F32, tag="ystage")
                        rstage = bst.tile([128, QC], F32, tag="rstage")
                        rt = bst.tile([128, QC], F32, tag="rt")
                        ycols = slice(jq * QC, (jq + 1) * QC)
                        nc.vector.tensor_copy(ystage[:], psY_t[:])
                        nc.vector.reciprocal_approx_fast(rstage[:], ystage[:])
                        if h == 1:  # [1|V]: rowsum on 0:64, y on 64:128
                            nc.sync.dma_start(rt[64:128, :], rstage[0:64, :])
                            nc.vector.tensor_mul(yT0[64:128, ycols],
                                                 ystage[64:128, :], rt[64:128, :])
                        else:       # [V|1]: y on 0:64, rowsum on 64:128
                            nc.sync.dma_start(rt[0:64, :], rstage[64:128, :])
                            dst = yT0[0:64, ycols] if h == 0 else yT1[:, ycols]
                            nc.vector.tensor_mul(dst, ystage[0:64, :], rt[0:64, :])

                    units = ([("P", kt) for kt in range(nkt)]
                             + [("H", i) for i in range(nu2)])
                    G = 4
                    groups = [units[g:g + G] for g in range(0, len(units), G)]

                    def do_av(u):
                        av_unit(u)
                        gu[0] += 1
                        if inserts and gu[0] % spacing == 0:
                            inserts.pop(0)()
                        if u == ("P", nkt - 1):
                            norm(0, psY["lo"])
                            norm(1, psY["hi"])
                        elif u == ("H", nu2 - 1):
                            norm(2, psY["h2"])

                    for u in groups[0]:
                        s_unit(u)
                    if tail_prev is not None:
                        tail_prev()
                    for gi, grp in enumerate(groups[:-1]):
                        for u in groups[gi + 1]:
                            s_unit(u)
                        for u in grp:
                            do_av(u)
                    # drain leftover inserts now: the next row's prologue S
                    # depends on this row's A-group (QKV) inserts, which must
                    # precede it in the PE FIFO or the queue deadlocks.
                    while inserts:
                        inserts.pop(0)()

                    def tail():
                        for u in groups[-1]:
                            do_av(u)
                    return tail

                # A(0): emit just the qk m-tiles for heads 0/1 and h2 up
                # front; the rest (v subs) slots in right after row 0's
                # S prologue so the first exp starts ~5us earlier.
                A0 = make_A_groups(0)
                A0[0]()
                A0[1]()
                A0[2]()
                A0_rest = A0[3:]
                # proj tiles are deferred to the late (ACT-saturated)
                # rows where the PE has slack; early rows are PE-bound on
                # the QKV inserts and would starve the scalar engine.
                PROJ_SCHED = {5: range(0, 6), 6: range(6, 14), 7: range(14, 28)}
                tail = lambda: [g() for g in A0_rest]
                for jq in range(NJQ):
                    inserts = []
                    if jq + 1 < NJQ:
                        inserts += make_A_groups(jq + 1)
                    inserts += [make_proj(tt) for tt in PROJ_SCHED.get(jq, [])]
                    tail = emit_B(jq, inserts, tail)
                tail()

                if debug:
                    nc.sync.dma_start(dq, qT[:])
                    nc.sync.dma_start(dk, kT[:])
                    nc.sync.dma_start(dv, vones[:])
                    nc.sync.dma_start(dy0, yT0[:])
                    nc.sync.dma_start(dy1, yT1[:])

                # remaining projection tiles (need row 7's yT)
                for tt in range(28, NTT):
                    make_proj(tt)()

    nc.compile()
    return nc


def _get_program():
    if "nc" not in _cache:
        _cache["nc"] = _build()
    return _cache["nc"]


def kernel(x, W_attn, b_attn, W_proj, b_proj):
    global last_results
    from concourse.bass_utils import run_bass_kernel_spmd

    x = np.asarray(x, dtype=np.float32)
    W_attn = np.asarray(W_attn, dtype=np.float32)
    b_attn = np.asarray(b_attn, dtype=np.float32)
    W_proj = np.asarray(W_proj, dtype=np.float32)
    b_proj = np.asarray(b_proj, dtype=np.float32)

    Wq, Wk, Wv = W_attn[:, 0:C], W_attn[:, C:2 * C], W_attn[:, 2 * C:3 * C]
    bq, bk, bv = b_attn[0:C], b_attn[C:2 * C], b_attn[2 * C:3 * C]
    scale = 1.0 / np.sqrt(D)

    xTb = [np.ascontiguousarray(x[b].T).astype(np.float16) for b in range(B)]
    tri = np.triu(np.ones((128, 128), dtype=np.float16))  # keep f >= p

    in_maps = []
    for core in range(NCORE):
        b = core // 4
        h0 = 3 * (core % 4)
        cs = slice(h0 * D, (h0 + HPC) * D)  # this core's 192 channels
        q_w = Wq[:, cs] * scale
        k_w = Wk[:, cs]
        # columns: [q0 q1 | k0 k1 | q2 k2]
        wqk_i = np.concatenate(
            [q_w[:, 0:128], k_w[:, 0:128], q_w[:, 128:192], k_w[:, 128:192]],
            axis=1)
        bq_c = bq[cs] * scale
        bk_c = bk[cs]
        bqk_i = np.zeros((128, 3), dtype=np.float32)
        bqk_i[:, 0] = bq_c[0:128]
        bqk_i[:, 1] = bk_c[0:128]
        bqk_i[0:64, 2] = bq_c[128:192]
        bqk_i[64:128, 2] = bk_c[128:192]
        in_maps.append({
            "xT": xTb[b],
            "wqk": wqk_i.astype(np.float16),
            "wv": np.ascontiguousarray(Wv[:, cs]).astype(np.float16),
            "wp": np.ascontiguousarray(W_proj[cs, :]).astype(np.float16),
            "bqk": bqk_i,
            "trimask": tri,
        })

    nc = _get_program()
    trace = os.environ.get("CC_ATTN_TRACE", "0") == "1"
    res = run_bass_kernel_spmd(nc, in_maps, core_ids=list(range(NCORE)),
                               trace=trace)
    last_results = res

    bias_row = (b_proj + bv @ W_proj).astype(np.float32)  # [768]
    out = np.empty((B, T, C), dtype=np.float32)
    for b in range(B):
        acc = res.results[4 * b]["z"].astype(np.float32).copy()
        for g in range(1, 4):
            acc += res.results[4 * b + g]["z"]
        out[b] = acc + bias_row
    return out
